# revision 18
# baseline (speedup 1.0000x reference)
"""DiffPool regression on 8 NeuronCores — hand-written Bass/Tile kernel.

Contract: kernel(**inputs) takes FULL unsharded numpy inputs (keys as in
setup_inputs()) and returns the FULL [8192, 1] float32 output.

Problem shape (hardcoded): B=8192 graphs, N=64 nodes/graph, C_IN=128,
HID=128, K=5, DEG=8; total nodes 524288, E=4194304, 8 cores.

Strategy
--------
Wall time is dominated by host->device transfer (axon tunnel), so:
  * algebra is collapsed on host:  W1/W2/W_lin fold into one vector
    w1v = W1 @ W2 @ W_lin, so the device only needs z = x @ [W_pool|w1v]
    ([64,6] per graph) from x plus the adjacency ops;
  * x is shipped as 24-bit fixed point (int16 hi + uint8 lo, scale 2^19,
    3 B/elem instead of 4) and reconstructed on device — validated to keep
    the output rel-err ~1e-3, far under the 2e-2 gate;
  * edges are shipped as two uint8 position arrays (src%64, dst%64).

Device kernel (per core, 1024 graphs as 512 partition-packed pairs):
  adjacency^T built by one-hot(is_equal) tiles contracted on the PE;
  GCN normalization via matmul row-sums + sqrt + reciprocal; softmax via
  fused exp+accum; DiffPool second stage collapses to a per-graph scalar.
"""

import os
import sys
import numpy as np
from concurrent.futures import ThreadPoolExecutor
from functools import partial

for _p in ("/opt/trn_rl_repo",):
    if _p not in sys.path and os.path.isdir(_p):
        sys.path.insert(0, _p)

B, N, C_IN, HID, K, DEG = 8192, 64, 128, 128, 5, 8
NCORES = 8
GPC = B // NCORES            # graphs per core = 1024
NPAIRS = GPC // 2            # pair of graphs per 128-partition tile = 512
NPC = GPC * N                # nodes per core = 65536
EPC = GPC * N * DEG          # edges per core = 524288
XSH = 19                     # x fixed-point shift (int24, scale 2^19)

_THREADS = min(16, os.cpu_count() or 8)


# ---------------------------------------------------------------------------
# Bass kernel builder (one core's program; SPMD-identical across cores)
# ---------------------------------------------------------------------------

def _build(nc, xhi, xlo, sp, dp, wc, bp, iot, kdg, one, cc, iden,
           npairs=NPAIRS, out=None):
    import concourse.bass as bass
    import concourse.mybir as mybir
    from concourse import tile
    from concourse.bass import ds

    f32 = mybir.dt.float32
    bf16 = mybir.dt.bfloat16
    AF = mybir.ActivationFunctionType
    OP = mybir.AluOpType
    AX = mybir.AxisListType

    if out is None:
        out = nc.dram_tensor("out", [npairs, 2], f32, kind="ExternalOutput")

    # DRAM views: pair-indexed
    xhi3 = xhi.rearrange("(a p) c -> a p c", p=128)        # [npairs,128,128]
    xlo3 = xlo.rearrange("(a p) c -> a p c", p=128)
    # edges: within a pair, edge index e = c*128 + p  (1024 edges/pair)
    sp3 = sp.rearrange("(a c p) -> a p c", p=128, c=8)     # [npairs,128,8]
    dp3 = dp.rearrange("(a c p) -> a p c", p=128, c=8)

    with tile.TileContext(nc) as tc:
        with (
            tc.tile_pool(name="consts", bufs=1) as cpool,
            tc.tile_pool(name="work", bufs=3) as wp,
            tc.tile_pool(name="oh", bufs=4) as ohp,
            tc.tile_pool(name="ps_xt", bufs=2, space="PSUM") as pxt,
            tc.tile_pool(name="ps_t", bufs=2, space="PSUM") as pt,
            tc.tile_pool(name="ps_s", bufs=3, space="PSUM") as psm,
        ):
            wc_t = cpool.tile_from(wc[:], name="wc_t")        # [128, 6] f32
            bp_t = cpool.tile_from(bp[:], name="bp_t")        # [128, 5]
            iot_t = cpool.tile_from(iot[:], name="iot_t")     # [128, 64]
            kdg_t = cpool.tile_from(kdg[:], name="kdg_t")     # [128, 64]
            one_t = cpool.tile_from(one[:], name="one_t")     # [128, 1]
            cc_t = cpool.tile_from(cc[:], name="cc_t")        # [128, 2]
            idn_t = cpool.tile_from(iden[:], name="idn_t")    # [128, 128] f32
            # complement of the pair-diagonal mask: nkd = (kdg == 0)
            nkd_t = cpool.tile([128, 64], f32, name="nkd_t")
            nc.vector.tensor_scalar(nkd_t[:], kdg_t[:], 0.0, None,
                                    op0=OP.is_equal)

            def body(pr):
                # ---- load + reconstruct x pair tile ([128 nodes, 128 ch])
                xhi_t = wp.tile([128, 128], mybir.dt.int16, name="xhi_t")
                xlo_t = wp.tile([128, 128], mybir.dt.uint8, name="xlo_t")
                nc.sync.dma_start(xhi_t[:], xhi3[ds(pr, 1)])
                nc.sync.dma_start(xlo_t[:], xlo3[ds(pr, 1)])
                xfh = wp.tile([128, 128], f32, name="xfh")
                xfl = wp.tile([128, 128], f32, name="xfl")
                xf = wp.tile([128, 128], f32, name="xf")
                nc.gpsimd.tensor_copy(xfh[:], xhi_t[:])
                nc.vector.tensor_scalar(xfl[:], xlo_t[:], 1.0 / 256.0, None,
                                        op0=OP.mult)
                nc.vector.tensor_tensor(xf[:], xfh[:], xfl[:], op=OP.add)
                # x^T via PE transpose
                xt_ps = pxt.tile([128, 128], f32, name="xt_ps")
                nc.tensor.transpose(xt_ps[:], xf[:], idn_t[:])
                xt = wp.tile([128, 128], f32, name="xt")
                nc.scalar.copy(xt[:], xt_ps[:])

                # ---- edges for the pair
                sp_t = wp.tile([128, 8], mybir.dt.uint8, name="sp_t")
                dp_t = wp.tile([128, 8], mybir.dt.uint8, name="dp_t")
                nc.sync.dma_start(sp_t[:], sp3[ds(pr, 1)])
                nc.sync.dma_start(dp_t[:], dp3[ds(pr, 1)])
                spf = wp.tile([128, 8], f32, name="spf")
                dpf = wp.tile([128, 8], f32, name="dpf")
                nc.gpsimd.tensor_copy(spf[:], sp_t[:])
                nc.gpsimd.tensor_copy(dpf[:], dp_t[:])

                # ---- small psum tile layout (one bank):
                #  z 0:6 | dsum 6:7 | c 7:8 | s6 8:14 | fin 14:16 |
                #  As 16:21 | P2 24:30
                sm = psm.tile([128, 32], f32, name="sm")

                # z = x @ Wc  -> [128 nodes, 6]
                nc.tensor.matmul(sm[:, 0:6], xt[:], wc_t[:],
                                 start=True, stop=True)

                # ---- adjacency^T: T[d, s] per graph (even rows 0:64,
                #      odd rows 64:128), accumulated over 4 edge chunks
                t_ps = pt.tile([128, 64], f32, name="t_ps")
                for g in (0, 1):
                    for c in range(4):
                        cc4 = g * 4 + c
                        ohS = ohp.tile([128, 64], bf16, name="ohS", tag="ohS")
                        ohD = ohp.tile([128, 64], bf16, name="ohD", tag="ohD")
                        nc.gpsimd.tensor_scalar(
                            ohS[:], iot_t[:], spf[:, cc4:cc4 + 1], None,
                            op0=OP.is_equal)
                        nc.vector.tensor_scalar(
                            ohD[:], iot_t[:], dpf[:, cc4:cc4 + 1], None,
                            op0=OP.is_equal)
                        nc.tensor.matmul(
                            t_ps[g * 64:g * 64 + 64, :], ohD[:], ohS[:],
                            start=(c == 0), stop=(c == 3),
                            skip_group_check=True)

                t_sb = wp.tile([128, 64], f32, name="t_sb")   # raw adj^T
                t_l = wp.tile([128, 64], f32, name="t_l")     # diag := 1
                nc.scalar.copy(t_sb[:], t_ps[:])
                nc.vector.tensor_tensor(t_l[:], t_ps[:], nkd_t[:], op=OP.mult)
                nc.vector.tensor_tensor(t_l[:], t_l[:], kdg_t[:], op=OP.add)

                # ---- GCN normalization: d = 1/sqrt(rowsum(adj_l))
                for g in (0, 1):
                    r = slice(g * 64, g * 64 + 64)
                    nc.tensor.matmul(sm[r, 6:7], t_l[r, :], one_t[r, :],
                                     start=True, stop=True,
                                     skip_group_check=True)
                dsq = wp.tile([128, 1], f32, name="dsq")
                dr = wp.tile([128, 1], f32, name="dr")
                nc.scalar.sqrt(dsq[:], sm[:, 6:7])
                nc.vector.reciprocal(dr[:], dsq[:])

                # zd = d * z ; s6 = d * (adj_l^T^T @ zd)
                zd = wp.tile([128, 6], f32, name="zd")
                nc.vector.tensor_scalar(zd[:], sm[:, 0:6], dr[:, 0:1], None,
                                        op0=OP.mult)
                for g in (0, 1):
                    r = slice(g * 64, g * 64 + 64)
                    nc.tensor.matmul(sm[r, 8:14], t_l[r, :], zd[r, :],
                                     start=True, stop=True,
                                     skip_group_check=True)
                s6f = wp.tile([128, 6], f32, name="s6f")
                nc.vector.tensor_scalar(s6f[:], sm[:, 8:14], dr[:, 0:1], None,
                                        op0=OP.mult)

                # ---- softmax over 5 cluster logits (+ b_pool)
                spre = wp.tile([128, 5], f32, name="spre")
                nc.vector.tensor_tensor(spre[:], s6f[:, 0:5], bp_t[:],
                                        op=OP.add)
                nm = wp.tile([128, 1], f32, name="nm")
                nc.vector.reduce_max(nm[:], spre[:], axis=AX.X, negate=True)
                e_t = wp.tile([128, 5], f32, name="e_t")
                rs_t = wp.tile([128, 1], f32, name="rs_t")
                nc.scalar.activation(e_t[:], spre[:], AF.Exp,
                                     bias=nm[:, 0:1], scale=1.0,
                                     accum_out=rs_t[:, 0:1])
                rr = wp.tile([128, 1], f32, name="rr")
                nc.vector.reciprocal(rr[:], rs_t[:])
                s_t = wp.tile([128, 5], f32, name="s_t")
                nc.vector.tensor_scalar(s_t[:], e_t[:], rr[:, 0:1], None,
                                        op0=OP.mult)

                # ---- y = s6[:,5] + c1 into Asy col 5; As = adj @ s
                asy = wp.tile([128, 6], f32, name="asy")
                nc.scalar.activation(asy[:, 5:6], s6f[:, 5:6], AF.Identity,
                                     bias=cc_t[:, 0:1], scale=1.0)
                for g in (0, 1):
                    r = slice(g * 64, g * 64 + 64)
                    nc.tensor.matmul(sm[r, 16:21], t_sb[r, :], s_t[r, :],
                                     start=True, stop=True,
                                     skip_group_check=True)
                nc.vector.tensor_copy(asy[:, 0:5], sm[:, 16:21])

                # ---- P2 = s^T @ [As | y] -> [5,6] per graph
                for g in (0, 1):
                    r = slice(g * 64, g * 64 + 64)
                    ro = slice(g * 64, g * 64 + 5)
                    nc.tensor.matmul(sm[ro, 24:30], s_t[r, :], asy[r, :],
                                     start=True, stop=True,
                                     skip_group_check=True)
                p2 = wp.tile([128, 6], f32, name="p2")
                t2l = wp.tile([128, 5], f32, name="t2l")
                r2s = wp.tile([128, 1], f32, name="r2s")
                d2 = wp.tile([128, 1], f32, name="d2")
                t2d = wp.tile([128, 5], f32, name="t2d")
                cf = wp.tile([128, 1], f32, name="cf")
                q = wp.tile([128, 1], f32, name="q")
                for g in (0, 1):
                    ro = slice(g * 64, g * 64 + 5)
                    nc.scalar.copy(p2[ro, :], sm[ro, 24:30])
                    nc.vector.tensor_tensor(t2l[ro, :], p2[ro, 0:5],
                                            nkd_t[ro, 0:5], op=OP.mult)
                    nc.vector.tensor_tensor(t2l[ro, :], t2l[ro, :],
                                            kdg_t[ro, 0:5], op=OP.add)
                    nc.vector.reduce_sum(r2s[ro, :], t2l[ro, :], axis=AX.X)
                    nc.scalar.sqrt(d2[ro, :], r2s[ro, :])
                    nc.vector.reciprocal(d2[ro, :], d2[ro, :])
                    nc.vector.tensor_scalar(t2d[ro, :], t2l[ro, :],
                                            d2[ro, 0:1], None, op0=OP.mult)
                    nc.tensor.matmul(sm[ro, 7:8], t2d[ro, :], one_t[ro, :],
                                     start=True, stop=True,
                                     skip_group_check=True)
                    nc.vector.tensor_tensor(cf[ro, :], sm[ro, 7:8],
                                            d2[ro, :], op=OP.mult)
                    nc.vector.tensor_tensor(q[ro, :], cf[ro, :],
                                            p2[ro, 5:6], op=OP.mult)
                    nc.tensor.matmul(sm[0:1, 14 + g:15 + g], q[ro, :],
                                     one_t[ro, :], start=True, stop=True,
                                     skip_group_check=True)

                # ---- out pair: + const, DMA to DRAM
                outt = wp.tile([1, 2], f32, name="outt")
                nc.scalar.activation(outt[:], sm[0:1, 14:16], AF.Identity,
                                     bias=cc_t[0:1, 1:2], scale=1.0)
                nc.sync.dma_start(out[ds(pr, 1)], outt[:])

            if npairs <= 8:
                for pr in range(npairs):
                    body(pr)
            else:
                tc.For_i_unrolled(0, npairs, 1, body, max_unroll=4)

    return (out,)


# ---------------------------------------------------------------------------
# Host side
# ---------------------------------------------------------------------------

def _consts(W_pool, b_pool, W1, b1, W2, b2, W_lin, b_lin):
    f64 = np.float64
    Wv = W2.astype(f64) @ W_lin.astype(f64)              # [128,1]
    w1v = W1.astype(f64) @ Wv                            # [128,1]
    c1 = (b1.astype(f64) @ Wv).item()
    const = (5.0 * (b2.astype(f64) @ W_lin.astype(f64)) + b_lin.astype(f64)).item()
    # x is (hi + lo/256) * 2^-(XSH-8); fold the scale into Wc
    Wc = np.concatenate([W_pool.astype(f64), w1v], axis=1)
    Wc = (Wc * (2.0 ** -(XSH - 8))).astype(np.float32)   # [128, 6]
    bp = np.broadcast_to(b_pool.astype(np.float32), (128, 5)).copy()
    iot = np.broadcast_to(np.arange(64, dtype=np.float32), (128, 64)).copy()
    p = np.arange(128)[:, None]
    kdg = (np.arange(64)[None, :] == (p % 64)).astype(np.float32)
    one = np.ones((128, 1), np.float32)
    cc = np.zeros((128, 2), np.float32)
    cc[:, 0] = c1
    cc[:, 1] = const
    iden = np.eye(128, dtype=np.float32)
    return Wc, bp, iot, kdg, one, cc, iden


def _pack_x_slice(xs_rows):
    """f32 rows -> (int16 hi, uint8 lo) 24-bit fixed point via byte views."""
    t = xs_rows * np.float32(1 << XSH)
    np.rint(t, out=t)
    np.clip(t, -(1 << 23), (1 << 23) - 1, out=t)
    xi = t.astype(np.int32)
    b = xi.view(np.uint8).reshape(-1, 4)          # little-endian bytes
    lo = np.ascontiguousarray(b[:, 0]).reshape(xs_rows.shape)
    hi = np.ascontiguousarray(b[:, 1:3]).view(np.int16).reshape(xs_rows.shape)
    return hi, lo


_CACHE = {}

_IN_NAMES = ["xhi", "xlo", "sp", "dp", "wc", "bp", "iot", "kdg", "one",
             "cc", "iden"]
_IN_SHAPES = [(NPC, 128), (NPC, 128), (EPC,), (EPC,), (128, 6), (128, 5),
              (128, 64), (128, 64), (128, 1), (128, 2), (128, 128)]


def _get_fn():
    """Build the Bass program once and wrap it in a cached
    jit(shard_map(bass_exec)) callable — the same construction
    run_bass_via_pjrt uses under axon, but reusable across calls."""
    if "fn" in _CACHE:
        return _CACHE["fn"]
    import jax
    from jax.sharding import Mesh, PartitionSpec
    from jax.experimental.shard_map import shard_map
    import concourse.bacc as bacc
    import concourse.mybir as mybir
    from concourse.bass2jax import (_bass_exec_p, install_neuronx_cc_hook,
                                    partition_id_tensor)

    install_neuronx_cc_hook()

    nc = bacc.Bacc("TRN2", target_bir_lowering=False, debug=False)
    dts = [mybir.dt.int16, mybir.dt.uint8, mybir.dt.uint8, mybir.dt.uint8] + \
          [mybir.dt.float32] * 7
    handles = [nc.dram_tensor(n, list(s), d, kind="ExternalInput")
               for n, s, d in zip(_IN_NAMES, _IN_SHAPES, dts)]
    _build(nc, *handles, npairs=NPAIRS)
    nc.finalize()

    part_name = nc.partition_id_tensor.name if nc.partition_id_tensor else None
    out_avals = (jax.core.ShapedArray((NPAIRS, 2), np.float32),)
    in_names = tuple(_IN_NAMES) + ("out",)
    if part_name is not None:
        in_names = in_names + (part_name,)

    def _body(*args):
        operands = list(args)
        if part_name is not None:
            operands.append(partition_id_tensor())
        outs = _bass_exec_p.bind(
            *operands,
            out_avals=out_avals,
            in_names=in_names,
            out_names=("out",),
            lowering_input_output_aliases=(),
            sim_require_finite=True,
            sim_require_nnan=True,
            nc=nc,
        )
        return tuple(outs)

    devices = jax.devices()[:NCORES]
    mesh = Mesh(np.asarray(devices), ("core",))
    n_args = len(_IN_NAMES) + 1  # + donated zero output buffer
    in_specs = (PartitionSpec("core"),) * n_args
    out_specs = (PartitionSpec("core"),)
    sharded = jax.jit(
        shard_map(_body, mesh=mesh, in_specs=in_specs, out_specs=out_specs,
                  check_rep=False),
        donate_argnums=(n_args - 1,),
        keep_unused=True,
    )
    _CACHE["nc"] = nc
    _CACHE["fn"] = sharded
    _CACHE["mesh"] = mesh
    _CACHE["devs"] = devices
    return sharded


def kernel(x, edge_index, batch, W_pool, b_pool, W1, b1, W2, b2, W_lin, b_lin,
           num_graphs, max_nodes):
    import jax
    from jax.sharding import NamedSharding, PartitionSpec

    x = np.asarray(x, dtype=np.float32)
    ei = np.asarray(edge_index)

    fn = _get_fn()
    mesh, devs = _CACHE["mesh"], _CACHE["devs"]
    shard = NamedSharding(mesh, PartitionSpec("core"))

    # tiny tensors first: weight-derived constants + donated output zeros
    Wc, bp, iot, kdg, one, cc, iden = _consts(
        np.asarray(W_pool, np.float32), np.asarray(b_pool, np.float32),
        np.asarray(W1, np.float32), np.asarray(b1, np.float32),
        np.asarray(W2, np.float32), np.asarray(b2, np.float32),
        np.asarray(W_lin, np.float32), np.asarray(b_lin, np.float32))
    reps_dev = [jax.device_put(np.tile(a, (NCORES, 1)), shard)
                for a in (Wc, bp, iot, kdg, one, cc, iden)]
    zeros = jax.device_put(np.zeros((NCORES * NPAIRS, 2), np.float32), shard)

    # pipeline: pack each core's slice, ship it immediately (async),
    # overlap packing of core c+1 with the wire transfer of core c
    hi_p, lo_p, sp_p, dp_p = [], [], [], []
    for c in range(NCORES):
        hi_c, lo_c = _pack_x_slice(x[c * NPC:(c + 1) * NPC])
        hi_p.append(jax.device_put(hi_c, devs[c]))
        lo_p.append(jax.device_put(lo_c, devs[c]))
        es = slice(c * EPC, (c + 1) * EPC)
        sp_p.append(jax.device_put((ei[0, es] & 63).astype(np.uint8), devs[c]))
        dp_p.append(jax.device_put((ei[1, es] & 63).astype(np.uint8), devs[c]))

    mk = jax.make_array_from_single_device_arrays
    hi = mk((NCORES * NPC, 128), shard, hi_p)
    lo = mk((NCORES * NPC, 128), shard, lo_p)
    sp = mk((NCORES * EPC,), shard, sp_p)
    dp = mk((NCORES * EPC,), shard, dp_p)

    out = fn(hi, lo, sp, dp, *reps_dev, zeros)
    return np.asarray(out[0], dtype=np.float32).reshape(B, 1)
